# revision 26
# baseline (speedup 1.0000x reference)
"""KV-cache attention (B=16,T=32,D=2048,H=16,DK=128,S=4096) on 8 TRN2 cores.

Sharding: Megatron-style tensor parallel over heads. Core c owns heads
{2c, 2c+1}: it gets the q/k/v weight rows for those heads, the k/v cache
slices, and computes attention + its out_proj partial (contraction over its
256 attn_flat columns). Host sums the 8 partials (the TP all-reduce
epilogue) and adds out_b.

v2: everything bf16 on the wire and on PE (4x PE throughput, 2x less DMA
traffic vs fp32), one DMA dispatch per tensor (host pre-packs per-partition-
contiguous layouts, incl. a fused k|v block per (b,h)), and attention
computed transposed: PV uses v-chunk-stationary matmuls so the result lands
directly as attnT [d, t] (no PE transposes), softmax denominator via
ones-stationary matmuls accumulated in PSUM, normalization via a rank-1
broadcast matmul + DVE multiply.
"""

import sys

for _p in ("/opt/trn_rl_repo",):
    if _p not in sys.path:
        sys.path.insert(0, _p)

import numpy as np
import ml_dtypes

import concourse.bass as bass
import concourse.bacc as bacc
import concourse.mybir as mybir
from concourse import tile
from concourse.bass_utils import run_bass_kernel_spmd

B, T, D = 16, 32, 2048
H, DK = 16, 128
S = 4096
NCORES = 8
HPC = H // NCORES            # heads per core = 2
NT = B * T                   # 512 tokens
QK = 2 * HPC * DK            # 512 q+k rows per core
VR = HPC * DK                # 256 v rows per core
SCALE = float(DK) ** -0.5
FP32 = mybir.dt.float32
BF16 = mybir.dt.bfloat16
AF = mybir.ActivationFunctionType
NKC = D // 128               # 16 contraction chunks for projections
NSC = S // 128               # 32 cache s-chunks per (b,h)
BF = ml_dtypes.bfloat16

_NC_CACHE = {}


KV_SPLIT = True  # alternate kv-cache DMA dispatches between SP and Pool
                 # queues so DGE/queue processing overlaps across engines


def _build_nc(loop_n=1, kv_split=None):
    if kv_split is None:
        kv_split = KV_SPLIT
    nc = bacc.Bacc()
    xTp = nc.dram_tensor("xTp", [128, NKC * NT], BF16, kind="ExternalInput")
    wqkp = nc.dram_tensor("wqkp", [128, NKC * QK], BF16, kind="ExternalInput")
    wvp = nc.dram_tensor("wvp", [128, NKC * VR], BF16, kind="ExternalInput")
    qkbp = nc.dram_tensor("qkbp", [128, 4], FP32, kind="ExternalInput")
    vbp = nc.dram_tensor("vbp", [1, VR], BF16, kind="ExternalInput")
    kvp = nc.dram_tensor("kvp", [B, 128, 2 * HPC * S], BF16, kind="ExternalInput")
    owp = nc.dram_tensor("owp", [128, HPC * D], BF16, kind="ExternalInput")
    outd = nc.dram_tensor("out", [NT, D], BF16, kind="ExternalOutput")

    from contextlib import ExitStack

    with tile.TileContext(nc) as tc:
        with (
            tc.tile_pool(name="resi", bufs=1) as resi,
            tc.tile_pool(name="kv", bufs=2) as kvpool,
            tc.tile_pool(name="expp", bufs=2) as expp,
            tc.tile_pool(name="small", bufs=2) as smallp,
            tc.tile_pool(name="outp", bufs=2) as outp,
            ExitStack() as loop_ctx,
        ):
            if loop_n > 1:
                # benchmark-only: run the whole body loop_n times inside the
                # NEFF so one dispatch measures loop_n executions
                loop_ctx.enter_context(tc.For_i(0, loop_n, 1))
            # ---- constants / small inputs ----
            ones_sb = resi.tile([128, 1], BF16, tag="ones")
            nc.vector.memset(ones_sb[:], 1.0)
            onesr_sb = resi.tile([1, 128], FP32, tag="onesr")
            nc.vector.memset(onesr_sb[:], 1.0)
            onesrb_sb = resi.tile([1, 128], BF16, tag="onesrb")
            nc.vector.memset(onesrb_sb[:], 1.0)
            qkb_sb = resi.tile([128, 4], FP32, tag="qkb")
            nc.sync.dma_start(qkb_sb[:], qkbp[:])
            vb_sb = resi.tile([1, VR], BF16, tag="vb")
            nc.sync.dma_start(vb_sb[:], vbp[:])

            # ---- bulk loads: one dispatch each ----
            xT_sb = resi.tile([128, NKC * NT], BF16, tag="xT")
            nc.sync.dma_start(xT_sb[:], xTp[:])
            wqk_sb = resi.tile([128, NKC * QK], BF16, tag="wqk")
            nc.sync.dma_start(wqk_sb[:], wqkp[:])
            wv_sb = resi.tile([128, NKC * VR], BF16, tag="wv")
            nc.sync.dma_start(wv_sb[:], wvp[:])
            ow_sb = resi.tile([128, HPC * D], BF16, tag="ow")
            nc.sync.dma_start(ow_sb[:], owp[:])

            # ---- phase 1: QKV projections ----
            # qkT[p, m*NT + t]: row m*128+p of (q_h0|q_h1|k_h0|k_h1), token t
            qkT = resi.tile([128, 4 * NT], BF16, tag="qkT")
            vnew = [
                resi.tile([T, VR], BF16, tag=f"vn{b}", name=f"vn{b}")
                for b in range(B)
            ]
            with tc.tile_pool(name="ps_q", bufs=2, space="PSUM") as ps_q:
                for m in range(4):
                    ps = ps_q.tile([128, NT], FP32, tag="qk_ps")
                    for kc in range(NKC):
                        nc.tensor.matmul(
                            ps[:],
                            wqk_sb[:, kc * QK + m * 128 : kc * QK + (m + 1) * 128],
                            xT_sb[:, kc * NT : (kc + 1) * NT],
                            start=(kc == 0),
                            stop=(kc == NKC - 1),
                        )
                    nc.vector.tensor_scalar_add(
                        qkT[:, m * NT : (m + 1) * NT], ps[:], qkb_sb[:, m : m + 1]
                    )
                for m in range(4):
                    ps = ps_q.tile([128, VR], FP32, tag="v_ps")
                    for kc in range(NKC):
                        nc.tensor.matmul(
                            ps[:],
                            xT_sb[:, kc * NT + m * 128 : kc * NT + m * 128 + 128],
                            wv_sb[:, kc * VR : (kc + 1) * VR],
                            start=(kc == 0),
                            stop=False,
                        )
                    # += 1 (x) vb so v_new rows carry the bias exactly
                    nc.tensor.matmul(
                        ps[:], onesrb_sb[:], vb_sb[:], start=False, stop=True
                    )
                    for r in range(4):
                        nc.vector.tensor_copy(
                            vnew[4 * m + r][:], ps[32 * r : 32 * r + 32, :]
                        )

            # ---- phase 2: attention per (b, h) pair, all transposed ----
            attnT = [
                resi.tile([128, NT], BF16, tag=f"at{h}", name=f"at{h}")
                for h in range(HPC)
            ]
            with (
                tc.tile_pool(name="ps_s", bufs=1, space="PSUM") as ps_s,
                tc.tile_pool(name="ps_pv", bufs=2, space="PSUM") as ps_pv,
                tc.tile_pool(name="ps_o", bufs=2, space="PSUM") as ps_o,
            ):
                for b in range(B):
                    # fused block for both heads: cols [h*S, (h+1)*S) =
                    # kT_h [d, s]; cols [2S + h*S, ...) = v_h as [j, d]
                    # chunks (s on partitions)
                    kv_sb = kvpool.tile([128, 2 * HPC * S], BF16, tag="kv")
                    kv_eng = nc.gpsimd if (kv_split and b % 2) else nc.sync
                    kv_eng.dma_start(kv_sb[:], kvp[b])
                    for h in range(HPC):
                        qT = qkT[:, h * NT + T * b : h * NT + T * b + T]
                        knT = qkT[:, (2 + h) * NT + T * b : (2 + h) * NT + T * b + T]

                        sA = ps_s.tile([128, 512], FP32, tag="sA")
                        sB = ps_s.tile([128, 512], FP32, tag="sB")
                        sC = ps_s.tile([T, T], FP32, tag="sC")
                        for j in range(NSC):
                            dst = sA if j < 16 else sB
                            col = (j % 16) * T
                            nc.tensor.matmul(
                                dst[:, col : col + T],
                                kv_sb[:, h * S + j * 128 : h * S + (j + 1) * 128],
                                qT,
                                start=True,
                                stop=True,
                            )
                        nc.tensor.matmul(sC[:], knT, qT, start=True, stop=True)

                        eA = expp.tile([128, 512], BF16, tag="eA")
                        eB = expp.tile([128, 512], BF16, tag="eB")
                        eC = expp.tile([T, T], BF16, tag="eC")
                        nc.scalar.activation(eA[:], sA[:], AF.Exp, scale=SCALE)
                        nc.scalar.activation(eB[:], sB[:], AF.Exp, scale=SCALE)
                        nc.scalar.activation(eC[:], sC[:], AF.Exp, scale=SCALE)

                        # one bank: cols 0:32 pvT, 32:64 den, 64:96 bcast
                        pvd = ps_pv.tile([128, 96], FP32, tag="pvd")
                        pv = pvd[:, 0:T]
                        den = pvd[0:1, T : 2 * T]
                        bc = pvd[:, 2 * T : 3 * T]
                        for j in range(NSC):
                            e_sl = (eA if j < 16 else eB)[
                                :, (j % 16) * T : (j % 16 + 1) * T
                            ]
                            nc.tensor.matmul(
                                pv,
                                kv_sb[:, (HPC + h) * S + j * 128
                                      : (HPC + h) * S + (j + 1) * 128],
                                e_sl,
                                start=(j == 0),
                                stop=False,
                            )
                        nc.tensor.matmul(
                            pv,
                            vnew[b][:, h * DK : (h + 1) * DK],
                            eC[:],
                            start=False,
                            stop=True,
                        )
                        # softmax denominator [1, t]
                        for j in range(NSC):
                            e_sl = (eA if j < 16 else eB)[
                                :, (j % 16) * T : (j % 16 + 1) * T
                            ]
                            nc.tensor.matmul(
                                den, ones_sb[:, 0:1], e_sl,
                                start=(j == 0), stop=False,
                            )
                        nc.tensor.matmul(
                            den, ones_sb[0:T, 0:1], eC[:],
                            start=False, stop=True,
                        )
                        rec = smallp.tile([1, T], FP32, tag="rec")
                        nc.vector.reciprocal(rec[:], den)
                        # broadcast rec across partitions via rank-1 matmul
                        nc.tensor.matmul(bc, onesr_sb[:], rec[:],
                                         start=True, stop=True)
                        bcs = smallp.tile([128, T], FP32, tag="bcs")
                        nc.scalar.activation(bcs[:], bc, AF.Copy)
                        nc.vector.tensor_mul(
                            attnT[h][:, T * b : T * b + T], pv, bcs[:]
                        )

                    # out_proj partial for token block m as soon as its
                    # batches (4m..4m+3) are done
                    if b % 4 == 3:
                        m = b // 4
                        ob = outp.tile([128, D], BF16, tag="ob")
                        for n in range(4):
                            ps = ps_o.tile([128, 512], FP32, tag="op")
                            for c in range(HPC):
                                nc.tensor.matmul(
                                    ps[:],
                                    attnT[c][:, m * 128 : (m + 1) * 128],
                                    ow_sb[:, c * D + n * 512 : c * D + (n + 1) * 512],
                                    start=(c == 0),
                                    stop=(c == HPC - 1),
                                )
                            nc.vector.tensor_copy(
                                ob[:, n * 512 : (n + 1) * 512], ps[:]
                            )
                        nc.sync.dma_start(
                            outd[m * 128 : (m + 1) * 128, :], ob[:]
                        )
    nc.finalize()
    return nc


def _get_nc():
    if "nc" not in _NC_CACHE:
        _NC_CACHE["nc"] = _build_nc()
    return _NC_CACHE["nc"]


def make_in_maps(x, k_cache, v_cache, qkv_w, qkv_b, out_w, out_b):
    x = np.asarray(x, np.float32)
    k_cache = np.asarray(k_cache, np.float32)
    v_cache = np.asarray(v_cache, np.float32)
    qkv_w = np.asarray(qkv_w, np.float32)
    qkv_b = np.asarray(qkv_b, np.float32)
    out_w = np.asarray(out_w, np.float32)

    # xTp[p, kc, t] = x[t, kc*128+p]
    xTp = np.ascontiguousarray(
        x.reshape(NT, D).T.reshape(NKC, 128, NT).transpose(1, 0, 2)
    ).astype(BF).reshape(128, NKC * NT)

    kb = k_cache.astype(BF)   # (B, H, S, DK)
    vb_ = v_cache.astype(BF)  # (B, H, S, DK)

    in_maps = []
    for c in range(NCORES):
        r0 = VR * c
        hs = slice(HPC * c, HPC * (c + 1))
        # weight rows: q_h0, q_h1, k_h0, k_h1 (128 each)
        w_rows = np.concatenate(
            [qkv_w[r0 : r0 + VR], qkv_w[D + r0 : D + r0 + VR]], 0
        )  # (512, 2048)
        wqkp = np.ascontiguousarray(
            w_rows.T.reshape(NKC, 128, QK).transpose(1, 0, 2)
        ).astype(BF).reshape(128, NKC * QK)
        wvp = np.ascontiguousarray(
            qkv_w[2 * D + r0 : 2 * D + r0 + VR].T
            .reshape(NKC, 128, VR).transpose(1, 0, 2)
        ).astype(BF).reshape(128, NKC * VR)
        qkbp = np.ascontiguousarray(
            np.concatenate([qkv_b[r0 : r0 + VR], qkv_b[D + r0 : D + r0 + VR]])
            .reshape(4, 128).T
        )
        vbp = qkv_b[2 * D + r0 : 2 * D + r0 + VR].astype(BF).reshape(1, VR)
        # kv block per b: [128, 4S] = [ kT_h0 | kT_h1 | v_h0 | v_h1 ],
        # kT_h = k[d, s], v_h = v as [j, d] chunks (s on partitions)
        kT = kb[:, hs].transpose(0, 3, 1, 2)                    # (B,128,2,S)
        vv = vb_[:, hs].reshape(B, HPC, NSC, 128, DK).transpose(0, 3, 1, 2, 4)
        kv = np.concatenate(
            [kT.reshape(B, 128, HPC * S), vv.reshape(B, 128, HPC * S)], axis=2
        )                                                        # (B,128,4S)
        owp = np.ascontiguousarray(
            out_w[:, r0 : r0 + VR].T.reshape(HPC, 128, D).transpose(1, 0, 2)
        ).astype(BF).reshape(128, HPC * D)
        in_maps.append(
            dict(xTp=xTp, wqkp=wqkp, wvp=wvp, qkbp=qkbp, vbp=vbp,
                 kvp=np.ascontiguousarray(kv), owp=owp)
        )
    return in_maps


def kernel(x, k_cache, v_cache, qkv_w, qkv_b, out_w, out_b):
    out_b = np.asarray(out_b, np.float32)
    in_maps = make_in_maps(x, k_cache, v_cache, qkv_w, qkv_b, out_w, out_b)
    nc = _get_nc()
    res = run_bass_kernel_spmd(nc, in_maps, list(range(NCORES))).results
    out = res[0]["out"].astype(np.float32)
    for c in range(1, NCORES):
        out = out + res[c]["out"].astype(np.float32)
    out = out + out_b[None, :]
    return out.reshape(B, T, D).astype(np.float32)


if __name__ == "__main__":
    rng = np.random.default_rng(0)
    ins = {
        "x": rng.standard_normal((B, T, D)).astype(np.float32),
        "k_cache": rng.standard_normal((B, H, S, DK)).astype(np.float32),
        "v_cache": rng.standard_normal((B, H, S, DK)).astype(np.float32),
        "qkv_w": (rng.standard_normal((3 * D, D)) / np.sqrt(D)).astype(np.float32),
        "qkv_b": np.zeros(3 * D, np.float32),
        "out_w": (rng.standard_normal((D, D)) / np.sqrt(D)).astype(np.float32),
        "out_b": np.zeros(D, np.float32),
    }
    o = kernel(**ins)
    print(o.shape, o.dtype, float(np.abs(o).max()))
